# revision 12
# baseline (speedup 1.0000x reference)
"""Trainium2 Bass kernel for the DAN classifier (gather + segment-mean + MLP + BCE).

Data-parallel across 8 NeuronCores: each core owns 512 whole sentences
(segments). Host splits the token stream at sentence boundaries, buckets each
core's tokens by (segment-group of 128, vocab-quarter of 25600) so embedding
row indices fit in int16 for the bulk dma_gather, and pads each bucket to a
common capacity. On device: dma_gather (4 parallel SWDGE queues, one per
vocab quarter - each queue's descriptors are generated by a different pair of
Q7 cores) pulls bf16 embedding rows from HBM; a one-hot(segment) matmul on
the TensorEngine accumulates per-segment sums in fp32 PSUM; the tiny MLP
head + BCE run on-chip; each core emits its partial loss. Host sums the 8
partials (the all-reduce of the scalar loss).
"""

import sys

try:
    import concourse  # noqa: F401
except ImportError:
    sys.path.insert(0, "/opt/trn_rl_repo")

import ml_dtypes
import numpy as np

import concourse.bass as bass
import concourse.tile as tile
from concourse import bacc, mybir
from concourse.bass_utils import run_bass_kernel_spmd

# Problem constants (hardcoded per harness contract).
V = 100000
H = 128
B = 4096
T = 409600
N_CORES = 8

SEGS_PER_CORE = B // N_CORES          # 512
N_GROUPS = 4                          # segment groups of 128 per core
N_QUARTERS = 4                        # vocab quarters
VQ = 25600                            # vocab rows per quarter (int16-safe)

F32 = mybir.dt.float32
BF16 = mybir.dt.bfloat16
I16 = mybir.dt.int16
BF16_NP = ml_dtypes.bfloat16


def _build(nc, c_sub, tiles_sb):
    """Emit the SPMD per-core graph. c_sub = padded tokens per sub-block."""
    n_tiles = c_sub // 128  # token tiles of 128 per sub-block
    n_sub = N_GROUPS * N_QUARTERS

    # ---- DRAM parameters (per-core shards arrive via in_maps) ----
    embed = nc.dram_tensor("embed", [V, H], BF16, kind="ExternalInput")
    idx_d = nc.dram_tensor("idx", [128, n_sub * (c_sub // 16)], I16,
                           kind="ExternalInput")
    seg_d = nc.dram_tensor("seg", [128, n_sub * n_tiles], BF16,
                           kind="ExternalInput")
    recip_d = nc.dram_tensor("recip", [128, N_GROUPS], F32, kind="ExternalInput")
    iota_d = nc.dram_tensor("iota", [128, 128], BF16, kind="ExternalInput")
    y_d = nc.dram_tensor("y", [1, SEGS_PER_CORE], F32, kind="ExternalInput")
    w_hid_d = nc.dram_tensor("w_hid", [H, H], F32, kind="ExternalInput")
    b_hid_d = nc.dram_tensor("b_hid", [H, 1], F32, kind="ExternalInput")
    w_out_d = nc.dram_tensor("w_out", [H, 1], F32, kind="ExternalInput")
    b_out_d = nc.dram_tensor("b_out", [1, 1], F32, kind="ExternalInput")
    out_d = nc.dram_tensor("out", [1, 1], F32, kind="ExternalOutput")

    with tile.TileContext(nc) as tc:
        with (
            tc.tile_pool(name="const", bufs=1) as cpool,
            tc.tile_pool(name="gather", bufs=3) as gpool,
            tc.tile_pool(name="onehot", bufs=4) as opool,
            tc.tile_pool(name="mlp", bufs=1) as mpool,
            tc.tile_pool(name="psum", bufs=2, space="PSUM") as ppool,
            tc.tile_pool(name="psum_mlp", bufs=1, space="PSUM") as pmpool,
        ):
            # ---- constants / metadata loads ----
            idx_sb = cpool.tile([128, n_sub * (c_sub // 16)], I16)
            seg_sb = cpool.tile([128, n_sub * n_tiles], BF16)
            gq = N_QUARTERS * (c_sub // 16)
            gs = N_QUARTERS * n_tiles
            for g in range(N_GROUPS):
                nc.sync.dma_start(out=idx_sb[:, g * gq : (g + 1) * gq],
                                  in_=idx_d[:, g * gq : (g + 1) * gq])
                nc.sync.dma_start(out=seg_sb[:, g * gs : (g + 1) * gs],
                                  in_=seg_d[:, g * gs : (g + 1) * gs])
            iota_sb = cpool.tile([128, 128], BF16)
            nc.sync.dma_start(out=iota_sb[:], in_=iota_d[:])
            recip_sb = cpool.tile([128, N_GROUPS], F32)
            nc.sync.dma_start(out=recip_sb[:], in_=recip_d[:])
            y_sb = cpool.tile([1, SEGS_PER_CORE], F32)
            nc.sync.dma_start(out=y_sb[:], in_=y_d[:])
            w_hid_sb = cpool.tile([H, H], F32)
            nc.sync.dma_start(out=w_hid_sb[:], in_=w_hid_d[:])
            b_hid_sb = cpool.tile([H, 1], F32)
            nc.sync.dma_start(out=b_hid_sb[:], in_=b_hid_d[:])
            w_out_sb = cpool.tile([H, 1], F32)
            nc.sync.dma_start(out=w_out_sb[:], in_=w_out_d[:])
            b_out_sb = cpool.tile([1, 1], F32)
            nc.sync.dma_start(out=b_out_sb[:], in_=b_out_d[:])

            from concourse.masks import make_identity
            ident_sb = cpool.tile([128, 128], F32)
            make_identity(nc, ident_sb[:])

            widx = cpool.tile([128, 1], I16)
            nc.gpsimd.memset(widx[:], 0)
            wg = cpool.tile([128, 1, 128], BF16)
            nc.gpsimd.dma_gather(wg[:], embed[0:2, :], widx[:], 16, 16, H,
                                 single_packet=False, queue_num=0)

            warm = cpool.tile([1, 1], F32)
            for fn in (mybir.ActivationFunctionType.Tanh,
                       mybir.ActivationFunctionType.Exp,
                       mybir.ActivationFunctionType.Ln):
                nc.scalar.activation(out=warm[:], in_=b_out_sb[0:1, 0:1],
                                     func=fn)

            sent_t = mpool.tile([128, SEGS_PER_CORE], F32)  # [H, seg]

            c16 = c_sub // 16
            for g in range(N_GROUPS):
                gt = gpool.tile([128, N_QUARTERS, n_tiles, 128], BF16,
                                tag="gather")
                for q in (1, 2, 3, 0):
                    sb = g * N_QUARTERS + q
                    qrows = min(VQ, V - q * VQ)
                    tsb = tiles_sb[sb]
                    t1 = tsb // 2
                    for (lo, hi) in ((0, t1), (t1, tsb)):
                        nidx = (hi - lo) * 128
                        nc.gpsimd.dma_gather(
                            gt[:, q, lo:hi, :],
                            embed[q * VQ : q * VQ + qrows, :],
                            idx_sb[:, sb * c16 + lo * 8 : sb * c16 + hi * 8],
                            nidx,
                            nidx,
                            H,
                            single_packet=False,
                            queue_num=q,
                        )

                psum_s = ppool.tile([128, H], F32, tag="psum_s")
                for q in range(N_QUARTERS):
                    sb = g * N_QUARTERS + q
                    tsb = tiles_sb[sb]
                    oh = opool.tile([128, n_tiles, 128], BF16, tag="onehot")
                    nc.vector.tensor_tensor(
                        out=oh[:, :tsb, :],
                        in0=seg_sb[:, sb * n_tiles : sb * n_tiles + tsb]
                        .rearrange("p (t u) -> p t u", u=1)
                        .to_broadcast([128, tsb, 128]),
                        in1=iota_sb[:]
                        .rearrange("p (u m) -> p u m", u=1)
                        .to_broadcast([128, tsb, 128]),
                        op=mybir.AluOpType.is_equal,
                    )
                    for j in range(tsb):
                        nc.tensor.matmul(
                            psum_s[:],
                            lhsT=oh[:, j, :],
                            rhs=gt[:, q, j, :],
                            start=(q == 0 and j == 0),
                            stop=(q == N_QUARTERS - 1 and j == tsb - 1),
                        )

                # segment means for this group: psum * (1/count)
                sent_g = mpool.tile([128, H], F32, tag="sent_g")
                nc.vector.tensor_scalar(
                    out=sent_g[:],
                    in0=psum_s[:],
                    scalar1=recip_sb[:, g : g + 1],
                    scalar2=None,
                    op0=mybir.AluOpType.mult,
                )
                # transpose [seg, H] -> [H, seg] chunk of sent_t
                psum_t = ppool.tile([128, 128], F32, tag="psum_t")
                nc.tensor.transpose(psum_t[:], sent_g[:], ident_sb[:])
                nc.vector.tensor_copy(
                    out=sent_t[:, g * 128 : (g + 1) * 128], in_=psum_t[:]
                )

            # ---- MLP head ----
            psum_hid = pmpool.tile([128, SEGS_PER_CORE], F32, tag="psum_hid")
            nc.tensor.matmul(psum_hid[:], lhsT=w_hid_sb[:], rhs=sent_t[:],
                             start=True, stop=True)
            hid = mpool.tile([128, SEGS_PER_CORE], F32)
            nc.scalar.activation(
                out=hid[:], in_=psum_hid[:],
                func=mybir.ActivationFunctionType.Tanh,
                bias=b_hid_sb[:, 0:1],
            )
            psum_p = pmpool.tile([1, SEGS_PER_CORE], F32, tag="psum_p")
            nc.tensor.matmul(psum_p[:], lhsT=w_out_sb[:], rhs=hid[:],
                             start=True, stop=True)
            ep = mpool.tile([1, SEGS_PER_CORE], F32)
            nc.scalar.activation(
                out=ep[:], in_=psum_p[:],
                func=mybir.ActivationFunctionType.Exp,
                bias=b_out_sb[0:1, 0:1],
            )
            sp = mpool.tile([1, SEGS_PER_CORE], F32)
            sp_sum = mpool.tile([1, 1], F32)
            nc.scalar.activation(
                out=sp[:], in_=ep[:],
                func=mybir.ActivationFunctionType.Ln,
                bias=1.0, accum_out=sp_sum[:],
            )
            x_sb = mpool.tile([1, SEGS_PER_CORE], F32)
            nc.vector.tensor_scalar(
                out=x_sb[:], in0=psum_p[:], scalar1=b_out_sb[0:1, 0:1],
                scalar2=None, op0=mybir.AluOpType.add,
            )
            yx = mpool.tile([1, SEGS_PER_CORE], F32)
            nc.vector.tensor_tensor(out=yx[:], in0=y_sb[:], in1=x_sb[:],
                                    op=mybir.AluOpType.mult)
            yx_sum = mpool.tile([1, 1], F32)
            nc.vector.tensor_reduce(out=yx_sum[:], in_=yx[:],
                                    axis=mybir.AxisListType.X,
                                    op=mybir.AluOpType.add)
            loss = mpool.tile([1, 1], F32)
            nc.vector.tensor_tensor(out=loss[:], in0=sp_sum[:], in1=yx_sum[:],
                                    op=mybir.AluOpType.subtract)
            nc.sync.dma_start(out=out_d[:], in_=loss[:])

    nc.compile()
    return nc


def _prep_inputs(token_ids, segment_ids, y_true, embed_table, W_hid, b_hid,
                 W_out, b_out):
    """Host-side shard + bucket + pad. Returns (c_sub, in_maps)."""
    token_ids = np.asarray(token_ids, dtype=np.int64)
    segment_ids = np.asarray(segment_ids, dtype=np.int64)
    y_true = np.asarray(y_true, dtype=np.float32)
    embed_bf16 = np.ascontiguousarray(
        np.asarray(embed_table, dtype=np.float32).astype(BF16_NP))

    # sentence-aligned core boundaries
    bounds = np.searchsorted(segment_ids, np.arange(0, B + 1, SEGS_PER_CORE))
    counts = np.bincount(segment_ids, minlength=B).astype(np.float32)
    recip_all = 1.0 / np.maximum(counts, 1.0)

    # bucket tokens per (core, group, quarter)
    per_core = []
    c_max = 0
    for c in range(N_CORES):
        lo, hi = bounds[c], bounds[c + 1]
        tid = token_ids[lo:hi]
        seg_loc = segment_ids[lo:hi] - c * SEGS_PER_CORE
        grp = seg_loc >> 7
        seg_in_grp = (seg_loc & 127).astype(np.float32)
        q = tid // VQ
        loc_idx = (tid - q * VQ).astype(np.int64)
        subs = []
        for g in range(N_GROUPS):
            for qq in range(N_QUARTERS):
                sel = (grp == g) & (q == qq)
                li, sg = loc_idx[sel], seg_in_grp[sel]
                order = np.argsort(li, kind="stable")
                subs.append((li[order], sg[order]))
                c_max = max(c_max, int(sel.sum()))
        per_core.append(subs)

    c_sub = ((c_max + 127) // 128) * 128
    n_tiles = c_sub // 128
    n_sub = N_GROUPS * N_QUARTERS
    sb_max = [0] * n_sub
    for c in range(N_CORES):
        for sbi, (li, sg) in enumerate(per_core[c]):
            sb_max[sbi] = max(sb_max[sbi], li.shape[0])
    tiles_sb = tuple((m + 127) // 128 for m in sb_max)

    iota = np.broadcast_to(np.arange(128, dtype=np.float32),
                           (128, 128)).astype(BF16_NP)
    in_maps = []
    for c in range(N_CORES):
        idx_arr = np.zeros((128, n_sub * (c_sub // 16)), dtype=np.int16)
        seg_arr = np.full((128, n_sub * n_tiles), -1.0, dtype=BF16_NP)
        for sbi, (li, sg) in enumerate(per_core[c]):
            n = li.shape[0]
            ip = np.zeros(c_sub, dtype=np.int16)
            ip[:n] = li
            sp = np.full(c_sub, -1.0, dtype=np.float32)
            sp[:n] = sg
            wrapped = ip.reshape(c_sub // 16, 16).T  # [16, c_sub//16]
            idx_arr[:, sbi * (c_sub // 16) : (sbi + 1) * (c_sub // 16)] = (
                np.tile(wrapped, (8, 1))
            )
            seg_arr[:, sbi * n_tiles : (sbi + 1) * n_tiles] = (
                sp.reshape(n_tiles, 128).T.astype(BF16_NP)
            )
        recip_c = (
            recip_all[c * SEGS_PER_CORE : (c + 1) * SEGS_PER_CORE]
            .reshape(N_GROUPS, 128)
            .T.copy()
        )
        in_maps.append({
            "embed": embed_bf16,
            "idx": idx_arr,
            "seg": seg_arr,
            "recip": recip_c,
            "iota": iota,
            "y": np.ascontiguousarray(
                y_true[c * SEGS_PER_CORE : (c + 1) * SEGS_PER_CORE]
            ).reshape(1, SEGS_PER_CORE),
            "w_hid": np.ascontiguousarray(np.asarray(W_hid, dtype=np.float32)),
            "b_hid": np.asarray(b_hid, dtype=np.float32).reshape(H, 1),
            "w_out": np.ascontiguousarray(np.asarray(W_out, dtype=np.float32)),
            "b_out": np.asarray(b_out, dtype=np.float32).reshape(1, 1),
        })
    return c_sub, tiles_sb, in_maps


_CACHE = {}


def _get_nc(c_sub, tiles_sb):
    key = (c_sub, tiles_sb)
    nc = _CACHE.get(key)
    if nc is None:
        nc = bacc.Bacc("TRN2", target_bir_lowering=False, debug=False,
                       num_devices=N_CORES, num_swdge_queues=N_QUARTERS)
        _build(nc, c_sub, tiles_sb)
        _CACHE[key] = nc
    return nc


def kernel(token_ids, segment_ids, y_true, embed_table, W_hid, b_hid, W_out,
           b_out, _trace=False, _trace_kwargs=None):
    c_sub, tiles_sb, in_maps = _prep_inputs(token_ids, segment_ids, y_true,
                                            embed_table, W_hid, b_hid, W_out,
                                            b_out)
    nc = _get_nc(c_sub, tiles_sb)
    res = run_bass_kernel_spmd(nc, in_maps, core_ids=list(range(N_CORES)),
                               trace=_trace, **(_trace_kwargs or {}))
    total = np.float64(0.0)
    for r in res.results:
        total += np.float64(r["out"][0, 0])
    out = np.array(np.float32(total))
    if _trace:
        return out, res
    return out


# revision 13
# speedup vs baseline: 1.0593x; 1.0593x over previous
"""Trainium2 Bass kernel for the DAN classifier (gather + segment-mean + MLP + BCE).

Data-parallel across 8 NeuronCores: each core owns 512 whole sentences
(segments). Host splits the token stream at sentence boundaries, buckets each
core's tokens by (segment-group of 128, vocab-quarter of 25600) so embedding
row indices fit in int16 for the bulk dma_gather, and pads each bucket to a
common capacity. On device: dma_gather (4 parallel SWDGE queues, one per
vocab quarter - each queue's descriptors are generated by a different pair of
Q7 cores) pulls bf16 embedding rows from HBM; a one-hot(segment) matmul on
the TensorEngine accumulates per-segment sums in fp32 PSUM; the tiny MLP
head + BCE run on-chip; each core emits its partial loss. Host sums the 8
partials (the all-reduce of the scalar loss).
"""

import sys

try:
    import concourse  # noqa: F401
except ImportError:
    sys.path.insert(0, "/opt/trn_rl_repo")

import ml_dtypes
import numpy as np

import concourse.bass as bass
import concourse.tile as tile
from concourse import bacc, mybir
from concourse.bass_utils import run_bass_kernel_spmd

# Problem constants (hardcoded per harness contract).
V = 100000
H = 128
B = 4096
T = 409600
N_CORES = 8

SEGS_PER_CORE = B // N_CORES          # 512
N_GROUPS = 4                          # segment groups of 128 per core
N_QUARTERS = 4                        # vocab quarters
VQ = 25600                            # vocab rows per quarter (int16-safe)

F32 = mybir.dt.float32
BF16 = mybir.dt.bfloat16
I16 = mybir.dt.int16
BF16_NP = ml_dtypes.bfloat16


def _build(nc, c_sub, tiles_sb):
    """Emit the SPMD per-core graph. c_sub = padded tokens per sub-block."""
    n_tiles = c_sub // 128  # token tiles of 128 per sub-block
    n_sub = N_GROUPS * N_QUARTERS

    # ---- DRAM parameters (per-core shards arrive via in_maps) ----
    embed = nc.dram_tensor("embed", [V, H], BF16, kind="ExternalInput")
    idx_d = nc.dram_tensor("idx", [128, n_sub * (c_sub // 16)], I16,
                           kind="ExternalInput")
    seg_d = nc.dram_tensor("seg", [128, n_sub * n_tiles], BF16,
                           kind="ExternalInput")
    recip_d = nc.dram_tensor("recip", [128, N_GROUPS], F32, kind="ExternalInput")
    iota_d = nc.dram_tensor("iota", [128, 128], BF16, kind="ExternalInput")
    y_d = nc.dram_tensor("y", [1, SEGS_PER_CORE], F32, kind="ExternalInput")
    w_hid_d = nc.dram_tensor("w_hid", [H, H], F32, kind="ExternalInput")
    b_hid_d = nc.dram_tensor("b_hid", [H, 1], F32, kind="ExternalInput")
    w_out_d = nc.dram_tensor("w_out", [H, 1], F32, kind="ExternalInput")
    b_out_d = nc.dram_tensor("b_out", [1, 1], F32, kind="ExternalInput")
    out_d = nc.dram_tensor("out", [1, 1], F32, kind="ExternalOutput")

    with tile.TileContext(nc) as tc:
        with (
            tc.tile_pool(name="const", bufs=1) as cpool,
            tc.tile_pool(name="gather", bufs=3) as gpool,
            tc.tile_pool(name="onehot", bufs=4) as opool,
            tc.tile_pool(name="mlp", bufs=1) as mpool,
            tc.tile_pool(name="psum", bufs=2, space="PSUM") as ppool,
            tc.tile_pool(name="psum_mlp", bufs=1, space="PSUM") as pmpool,
        ):
            # ---- constants / metadata loads ----
            idx_sb = cpool.tile([128, n_sub * (c_sub // 16)], I16)
            seg_sb = cpool.tile([128, n_sub * n_tiles], BF16)
            gq = N_QUARTERS * (c_sub // 16)
            gs = N_QUARTERS * n_tiles
            for g in range(N_GROUPS):
                nc.sync.dma_start(out=idx_sb[:, g * gq : (g + 1) * gq],
                                  in_=idx_d[:, g * gq : (g + 1) * gq])
                nc.sync.dma_start(out=seg_sb[:, g * gs : (g + 1) * gs],
                                  in_=seg_d[:, g * gs : (g + 1) * gs])
            iota_sb = cpool.tile([128, 128], BF16)
            nc.sync.dma_start(out=iota_sb[:], in_=iota_d[:])
            recip_sb = cpool.tile([128, N_GROUPS], F32)
            nc.sync.dma_start(out=recip_sb[:], in_=recip_d[:])
            y_sb = cpool.tile([1, SEGS_PER_CORE], F32)
            nc.sync.dma_start(out=y_sb[:], in_=y_d[:])
            w_hid_sb = cpool.tile([H, H], F32)
            nc.sync.dma_start(out=w_hid_sb[:], in_=w_hid_d[:])
            b_hid_sb = cpool.tile([H, 1], F32)
            nc.sync.dma_start(out=b_hid_sb[:], in_=b_hid_d[:])
            w_out_sb = cpool.tile([H, 1], F32)
            nc.sync.dma_start(out=w_out_sb[:], in_=w_out_d[:])
            b_out_sb = cpool.tile([1, 1], F32)
            nc.sync.dma_start(out=b_out_sb[:], in_=b_out_d[:])

            from concourse.masks import make_identity
            ident_sb = cpool.tile([128, 128], F32)
            make_identity(nc, ident_sb[:])

            widx = cpool.tile([128, 1], I16)
            nc.gpsimd.memset(widx[:], 0)
            wg = cpool.tile([128, 1, 128], BF16)
            warm_g = nc.gpsimd.dma_gather(wg[:], embed[0:2, :], widx[:], 16,
                                          16, H, single_packet=False,
                                          queue_num=0)

            warm = cpool.tile([1, 1], F32)
            for fn in (mybir.ActivationFunctionType.Tanh,
                       mybir.ActivationFunctionType.Exp,
                       mybir.ActivationFunctionType.Ln):
                nc.scalar.activation(out=warm[:], in_=b_out_sb[0:1, 0:1],
                                     func=fn)

            sent_t = mpool.tile([128, SEGS_PER_CORE], F32)  # [H, seg]

            c16 = c_sub // 16
            for g in range(N_GROUPS):
                gt = gpool.tile([128, N_QUARTERS, n_tiles, 128], BF16,
                                tag="gather")
                for q in (1, 2, 3, 0):
                    sb = g * N_QUARTERS + q
                    qrows = min(VQ, V - q * VQ)
                    nidx = tiles_sb[sb] * 128
                    gi = nc.gpsimd.dma_gather(
                        gt[:, q, : tiles_sb[sb], :],
                        embed[q * VQ : q * VQ + qrows, :],
                        idx_sb[:, sb * c16 : sb * c16 + nidx // 16],
                        nidx,
                        nidx,
                        H,
                        single_packet=False,
                        queue_num=q,
                    )
                    tile.add_dep_helper(gi.ins, warm_g.ins, sync=False,
                                        reason="warm IRAM first")

                psum_s = ppool.tile([128, H], F32, tag="psum_s")
                for q in range(N_QUARTERS):
                    sb = g * N_QUARTERS + q
                    tsb = tiles_sb[sb]
                    oh = opool.tile([128, n_tiles, 128], BF16, tag="onehot")
                    nc.vector.tensor_tensor(
                        out=oh[:, :tsb, :],
                        in0=seg_sb[:, sb * n_tiles : sb * n_tiles + tsb]
                        .rearrange("p (t u) -> p t u", u=1)
                        .to_broadcast([128, tsb, 128]),
                        in1=iota_sb[:]
                        .rearrange("p (u m) -> p u m", u=1)
                        .to_broadcast([128, tsb, 128]),
                        op=mybir.AluOpType.is_equal,
                    )
                    for j in range(tsb):
                        nc.tensor.matmul(
                            psum_s[:],
                            lhsT=oh[:, j, :],
                            rhs=gt[:, q, j, :],
                            start=(q == 0 and j == 0),
                            stop=(q == N_QUARTERS - 1 and j == tsb - 1),
                        )

                # segment means for this group: psum * (1/count)
                sent_g = mpool.tile([128, H], F32, tag="sent_g")
                nc.vector.tensor_scalar(
                    out=sent_g[:],
                    in0=psum_s[:],
                    scalar1=recip_sb[:, g : g + 1],
                    scalar2=None,
                    op0=mybir.AluOpType.mult,
                )
                # transpose [seg, H] -> [H, seg] chunk of sent_t
                psum_t = ppool.tile([128, 128], F32, tag="psum_t")
                nc.tensor.transpose(psum_t[:], sent_g[:], ident_sb[:])
                nc.vector.tensor_copy(
                    out=sent_t[:, g * 128 : (g + 1) * 128], in_=psum_t[:]
                )

            # ---- MLP head ----
            psum_hid = pmpool.tile([128, SEGS_PER_CORE], F32, tag="psum_hid")
            nc.tensor.matmul(psum_hid[:], lhsT=w_hid_sb[:], rhs=sent_t[:],
                             start=True, stop=True)
            hid = mpool.tile([128, SEGS_PER_CORE], F32)
            nc.scalar.activation(
                out=hid[:], in_=psum_hid[:],
                func=mybir.ActivationFunctionType.Tanh,
                bias=b_hid_sb[:, 0:1],
            )
            psum_p = pmpool.tile([1, SEGS_PER_CORE], F32, tag="psum_p")
            nc.tensor.matmul(psum_p[:], lhsT=w_out_sb[:], rhs=hid[:],
                             start=True, stop=True)
            ep = mpool.tile([1, SEGS_PER_CORE], F32)
            nc.scalar.activation(
                out=ep[:], in_=psum_p[:],
                func=mybir.ActivationFunctionType.Exp,
                bias=b_out_sb[0:1, 0:1],
            )
            sp = mpool.tile([1, SEGS_PER_CORE], F32)
            sp_sum = mpool.tile([1, 1], F32)
            nc.scalar.activation(
                out=sp[:], in_=ep[:],
                func=mybir.ActivationFunctionType.Ln,
                bias=1.0, accum_out=sp_sum[:],
            )
            x_sb = mpool.tile([1, SEGS_PER_CORE], F32)
            nc.vector.tensor_scalar(
                out=x_sb[:], in0=psum_p[:], scalar1=b_out_sb[0:1, 0:1],
                scalar2=None, op0=mybir.AluOpType.add,
            )
            yx = mpool.tile([1, SEGS_PER_CORE], F32)
            nc.vector.tensor_tensor(out=yx[:], in0=y_sb[:], in1=x_sb[:],
                                    op=mybir.AluOpType.mult)
            yx_sum = mpool.tile([1, 1], F32)
            nc.vector.tensor_reduce(out=yx_sum[:], in_=yx[:],
                                    axis=mybir.AxisListType.X,
                                    op=mybir.AluOpType.add)
            loss = mpool.tile([1, 1], F32)
            nc.vector.tensor_tensor(out=loss[:], in0=sp_sum[:], in1=yx_sum[:],
                                    op=mybir.AluOpType.subtract)
            nc.sync.dma_start(out=out_d[:], in_=loss[:])

    nc.compile()
    return nc


def _prep_inputs(token_ids, segment_ids, y_true, embed_table, W_hid, b_hid,
                 W_out, b_out):
    """Host-side shard + bucket + pad. Returns (c_sub, in_maps)."""
    token_ids = np.asarray(token_ids, dtype=np.int64)
    segment_ids = np.asarray(segment_ids, dtype=np.int64)
    y_true = np.asarray(y_true, dtype=np.float32)
    embed_bf16 = np.ascontiguousarray(
        np.asarray(embed_table, dtype=np.float32).astype(BF16_NP))

    # sentence-aligned core boundaries
    bounds = np.searchsorted(segment_ids, np.arange(0, B + 1, SEGS_PER_CORE))
    counts = np.bincount(segment_ids, minlength=B).astype(np.float32)
    recip_all = 1.0 / np.maximum(counts, 1.0)

    # bucket tokens per (core, group, quarter)
    per_core = []
    c_max = 0
    for c in range(N_CORES):
        lo, hi = bounds[c], bounds[c + 1]
        tid = token_ids[lo:hi]
        seg_loc = segment_ids[lo:hi] - c * SEGS_PER_CORE
        grp = seg_loc >> 7
        seg_in_grp = (seg_loc & 127).astype(np.float32)
        q = tid // VQ
        loc_idx = (tid - q * VQ).astype(np.int64)
        subs = []
        for g in range(N_GROUPS):
            for qq in range(N_QUARTERS):
                sel = (grp == g) & (q == qq)
                li, sg = loc_idx[sel], seg_in_grp[sel]
                order = np.argsort(li, kind="stable")
                subs.append((li[order], sg[order]))
                c_max = max(c_max, int(sel.sum()))
        per_core.append(subs)

    c_sub = ((c_max + 127) // 128) * 128
    n_tiles = c_sub // 128
    n_sub = N_GROUPS * N_QUARTERS
    sb_max = [0] * n_sub
    for c in range(N_CORES):
        for sbi, (li, sg) in enumerate(per_core[c]):
            sb_max[sbi] = max(sb_max[sbi], li.shape[0])
    tiles_sb = tuple((m + 127) // 128 for m in sb_max)

    iota = np.broadcast_to(np.arange(128, dtype=np.float32),
                           (128, 128)).astype(BF16_NP)
    in_maps = []
    for c in range(N_CORES):
        idx_arr = np.zeros((128, n_sub * (c_sub // 16)), dtype=np.int16)
        seg_arr = np.full((128, n_sub * n_tiles), -1.0, dtype=BF16_NP)
        for sbi, (li, sg) in enumerate(per_core[c]):
            n = li.shape[0]
            ip = np.zeros(c_sub, dtype=np.int16)
            ip[:n] = li
            sp = np.full(c_sub, -1.0, dtype=np.float32)
            sp[:n] = sg
            wrapped = ip.reshape(c_sub // 16, 16).T  # [16, c_sub//16]
            idx_arr[:, sbi * (c_sub // 16) : (sbi + 1) * (c_sub // 16)] = (
                np.tile(wrapped, (8, 1))
            )
            seg_arr[:, sbi * n_tiles : (sbi + 1) * n_tiles] = (
                sp.reshape(n_tiles, 128).T.astype(BF16_NP)
            )
        recip_c = (
            recip_all[c * SEGS_PER_CORE : (c + 1) * SEGS_PER_CORE]
            .reshape(N_GROUPS, 128)
            .T.copy()
        )
        in_maps.append({
            "embed": embed_bf16,
            "idx": idx_arr,
            "seg": seg_arr,
            "recip": recip_c,
            "iota": iota,
            "y": np.ascontiguousarray(
                y_true[c * SEGS_PER_CORE : (c + 1) * SEGS_PER_CORE]
            ).reshape(1, SEGS_PER_CORE),
            "w_hid": np.ascontiguousarray(np.asarray(W_hid, dtype=np.float32)),
            "b_hid": np.asarray(b_hid, dtype=np.float32).reshape(H, 1),
            "w_out": np.ascontiguousarray(np.asarray(W_out, dtype=np.float32)),
            "b_out": np.asarray(b_out, dtype=np.float32).reshape(1, 1),
        })
    return c_sub, tiles_sb, in_maps


_CACHE = {}


def _get_nc(c_sub, tiles_sb):
    key = (c_sub, tiles_sb)
    nc = _CACHE.get(key)
    if nc is None:
        nc = bacc.Bacc("TRN2", target_bir_lowering=False, debug=False,
                       num_devices=N_CORES, num_swdge_queues=N_QUARTERS)
        _build(nc, c_sub, tiles_sb)
        _CACHE[key] = nc
    return nc


def kernel(token_ids, segment_ids, y_true, embed_table, W_hid, b_hid, W_out,
           b_out, _trace=False, _trace_kwargs=None):
    c_sub, tiles_sb, in_maps = _prep_inputs(token_ids, segment_ids, y_true,
                                            embed_table, W_hid, b_hid, W_out,
                                            b_out)
    nc = _get_nc(c_sub, tiles_sb)
    res = run_bass_kernel_spmd(nc, in_maps, core_ids=list(range(N_CORES)),
                               trace=_trace, **(_trace_kwargs or {}))
    total = np.float64(0.0)
    for r in res.results:
        total += np.float64(r["out"][0, 0])
    out = np.array(np.float32(total))
    if _trace:
        return out, res
    return out


# revision 14
# speedup vs baseline: 1.1197x; 1.0570x over previous
"""Trainium2 Bass kernel for the DAN classifier (gather + segment-mean + MLP + BCE).

Data-parallel across 8 NeuronCores: each core owns 512 whole sentences
(segments). Host splits the token stream at sentence boundaries, buckets each
core's tokens by (segment-group of 128, vocab-quarter of 25600) so embedding
row indices fit in int16 for the bulk dma_gather, and pads each bucket to a
common capacity. On device: dma_gather (4 parallel SWDGE queues, one per
vocab quarter - each queue's descriptors are generated by a different pair of
Q7 cores) pulls bf16 embedding rows from HBM; a one-hot(segment) matmul on
the TensorEngine accumulates per-segment sums in fp32 PSUM; the tiny MLP
head + BCE run on-chip; each core emits its partial loss. Host sums the 8
partials (the all-reduce of the scalar loss).
"""

import sys

try:
    import concourse  # noqa: F401
except ImportError:
    sys.path.insert(0, "/opt/trn_rl_repo")

import ml_dtypes
import numpy as np

import concourse.bass as bass
import concourse.tile as tile
from concourse import bacc, mybir
from concourse.bass_utils import run_bass_kernel_spmd

# Problem constants (hardcoded per harness contract).
V = 100000
H = 128
B = 4096
T = 409600
N_CORES = 8

SEGS_PER_CORE = B // N_CORES          # 512
# Tapered segment groups: early groups big (pipeline fill), last group small
# so the final async descriptor-generation lag is short.
GROUP_SEGS = (128, 128, 128, 96, 32)
GROUP_STARTS = (0, 128, 256, 384, 480)
N_GROUPS = len(GROUP_SEGS)
N_QUARTERS = 4                        # vocab quarters
VQ = 25600                            # vocab rows per quarter (int16-safe)

F32 = mybir.dt.float32
BF16 = mybir.dt.bfloat16
I16 = mybir.dt.int16
BF16_NP = ml_dtypes.bfloat16


def _build(nc, c_sub, tiles_sb):
    """Emit the SPMD per-core graph. c_sub = padded tokens per sub-block."""
    n_tiles = c_sub // 128  # token tiles of 128 per sub-block
    n_sub = N_GROUPS * N_QUARTERS

    # ---- DRAM parameters (per-core shards arrive via in_maps) ----
    embed = nc.dram_tensor("embed", [V, H], BF16, kind="ExternalInput")
    idx_d = nc.dram_tensor("idx", [128, n_sub * (c_sub // 16)], I16,
                           kind="ExternalInput")
    seg_d = nc.dram_tensor("seg", [128, n_sub * n_tiles], BF16,
                           kind="ExternalInput")
    recip_d = nc.dram_tensor("recip", [128, N_GROUPS], F32, kind="ExternalInput")
    iota_d = nc.dram_tensor("iota", [128, 128], BF16, kind="ExternalInput")
    y_d = nc.dram_tensor("y", [1, SEGS_PER_CORE], F32, kind="ExternalInput")
    w_hid_d = nc.dram_tensor("w_hid", [H, H], F32, kind="ExternalInput")
    b_hid_d = nc.dram_tensor("b_hid", [H, 1], F32, kind="ExternalInput")
    w_out_d = nc.dram_tensor("w_out", [H, 1], F32, kind="ExternalInput")
    b_out_d = nc.dram_tensor("b_out", [1, 1], F32, kind="ExternalInput")
    out_d = nc.dram_tensor("out", [1, 1], F32, kind="ExternalOutput")

    with tile.TileContext(nc) as tc:
        with (
            tc.tile_pool(name="const", bufs=1) as cpool,
            tc.tile_pool(name="gather", bufs=3) as gpool,
            tc.tile_pool(name="onehot", bufs=4) as opool,
            tc.tile_pool(name="mlp", bufs=1) as mpool,
            tc.tile_pool(name="psum", bufs=2, space="PSUM") as ppool,
            tc.tile_pool(name="psum_mlp", bufs=1, space="PSUM") as pmpool,
        ):
            # ---- constants / metadata loads ----
            idx_sb = cpool.tile([128, n_sub * (c_sub // 16)], I16)
            seg_sb = cpool.tile([128, n_sub * n_tiles], BF16)
            gq = N_QUARTERS * (c_sub // 16)
            gs = N_QUARTERS * n_tiles
            for g in range(N_GROUPS):
                nc.sync.dma_start(out=idx_sb[:, g * gq : (g + 1) * gq],
                                  in_=idx_d[:, g * gq : (g + 1) * gq])
                nc.sync.dma_start(out=seg_sb[:, g * gs : (g + 1) * gs],
                                  in_=seg_d[:, g * gs : (g + 1) * gs])
            iota_sb = cpool.tile([128, 128], BF16)
            nc.sync.dma_start(out=iota_sb[:], in_=iota_d[:])
            recip_sb = cpool.tile([128, N_GROUPS], F32)
            nc.sync.dma_start(out=recip_sb[:], in_=recip_d[:])
            y_sb = cpool.tile([1, SEGS_PER_CORE], F32)
            nc.sync.dma_start(out=y_sb[:], in_=y_d[:])
            w_hid_sb = cpool.tile([H, H], F32)
            nc.sync.dma_start(out=w_hid_sb[:], in_=w_hid_d[:])
            b_hid_sb = cpool.tile([H, 1], F32)
            nc.sync.dma_start(out=b_hid_sb[:], in_=b_hid_d[:])
            w_out_sb = cpool.tile([H, 1], F32)
            nc.sync.dma_start(out=w_out_sb[:], in_=w_out_d[:])
            b_out_sb = cpool.tile([1, 1], F32)
            nc.sync.dma_start(out=b_out_sb[:], in_=b_out_d[:])

            from concourse.masks import make_identity
            ident_sb = cpool.tile([128, 128], F32)
            make_identity(nc, ident_sb[:])

            widx = cpool.tile([128, 1], I16)
            nc.gpsimd.memset(widx[:], 0)
            wg = cpool.tile([128, 1, 128], BF16)
            warm_g = nc.gpsimd.dma_gather(wg[:], embed[0:2, :], widx[:], 16,
                                          16, H, single_packet=False,
                                          queue_num=0)

            warm = cpool.tile([1, 1], F32)
            for fn in (mybir.ActivationFunctionType.Tanh,
                       mybir.ActivationFunctionType.Exp,
                       mybir.ActivationFunctionType.Ln):
                nc.scalar.activation(out=warm[:], in_=b_out_sb[0:1, 0:1],
                                     func=fn)

            sent_t = mpool.tile([128, SEGS_PER_CORE], F32)  # [H, seg]

            c16 = c_sub // 16
            for g in range(N_GROUPS):
                gt = gpool.tile([128, N_QUARTERS, n_tiles, 128], BF16,
                                tag="gather")
                for q in (1, 2, 3, 0):
                    sb = g * N_QUARTERS + q
                    qrows = min(VQ, V - q * VQ)
                    nidx = tiles_sb[sb] * 128
                    gi = nc.gpsimd.dma_gather(
                        gt[:, q, : tiles_sb[sb], :],
                        embed[q * VQ : q * VQ + qrows, :],
                        idx_sb[:, sb * c16 : sb * c16 + nidx // 16],
                        nidx,
                        nidx,
                        H,
                        single_packet=False,
                        queue_num=q,
                    )
                    tile.add_dep_helper(gi.ins, warm_g.ins, sync=False,
                                        reason="warm IRAM first")

                psum_s = ppool.tile([128, H], F32, tag="psum_s")
                for q in range(N_QUARTERS):
                    sb = g * N_QUARTERS + q
                    tsb = tiles_sb[sb]
                    oh = opool.tile([128, n_tiles, 128], BF16, tag="onehot")
                    nc.vector.tensor_tensor(
                        out=oh[:, :tsb, :],
                        in0=seg_sb[:, sb * n_tiles : sb * n_tiles + tsb]
                        .rearrange("p (t u) -> p t u", u=1)
                        .to_broadcast([128, tsb, 128]),
                        in1=iota_sb[:]
                        .rearrange("p (u m) -> p u m", u=1)
                        .to_broadcast([128, tsb, 128]),
                        op=mybir.AluOpType.is_equal,
                    )
                    for j in range(tsb):
                        nc.tensor.matmul(
                            psum_s[:],
                            lhsT=oh[:, j, :],
                            rhs=gt[:, q, j, :],
                            start=(q == 0 and j == 0),
                            stop=(q == N_QUARTERS - 1 and j == tsb - 1),
                        )

                # segment means for this group: psum * (1/count)
                gstart, gsize = GROUP_STARTS[g], GROUP_SEGS[g]
                sent_g = mpool.tile([128, H], F32, tag="sent_g")
                nc.vector.tensor_scalar(
                    out=sent_g[:],
                    in0=psum_s[:],
                    scalar1=recip_sb[:, g : g + 1],
                    scalar2=None,
                    op0=mybir.AluOpType.mult,
                )
                # transpose [seg, H] -> [H, seg] chunk of sent_t
                psum_t = ppool.tile([128, 128], F32, tag="psum_t")
                nc.tensor.transpose(psum_t[:], sent_g[:], ident_sb[:])
                nc.vector.tensor_copy(
                    out=sent_t[:, gstart : gstart + gsize],
                    in_=psum_t[:, :gsize],
                )

            # ---- MLP head ----
            psum_hid = pmpool.tile([128, SEGS_PER_CORE], F32, tag="psum_hid")
            nc.tensor.matmul(psum_hid[:], lhsT=w_hid_sb[:], rhs=sent_t[:],
                             start=True, stop=True)
            hid = mpool.tile([128, SEGS_PER_CORE], F32)
            nc.scalar.activation(
                out=hid[:], in_=psum_hid[:],
                func=mybir.ActivationFunctionType.Tanh,
                bias=b_hid_sb[:, 0:1],
            )
            psum_p = pmpool.tile([1, SEGS_PER_CORE], F32, tag="psum_p")
            nc.tensor.matmul(psum_p[:], lhsT=w_out_sb[:], rhs=hid[:],
                             start=True, stop=True)
            ep = mpool.tile([1, SEGS_PER_CORE], F32)
            nc.scalar.activation(
                out=ep[:], in_=psum_p[:],
                func=mybir.ActivationFunctionType.Exp,
                bias=b_out_sb[0:1, 0:1],
            )
            sp = mpool.tile([1, SEGS_PER_CORE], F32)
            sp_sum = mpool.tile([1, 1], F32)
            nc.scalar.activation(
                out=sp[:], in_=ep[:],
                func=mybir.ActivationFunctionType.Ln,
                bias=1.0, accum_out=sp_sum[:],
            )
            x_sb = mpool.tile([1, SEGS_PER_CORE], F32)
            nc.vector.tensor_scalar(
                out=x_sb[:], in0=psum_p[:], scalar1=b_out_sb[0:1, 0:1],
                scalar2=None, op0=mybir.AluOpType.add,
            )
            yx = mpool.tile([1, SEGS_PER_CORE], F32)
            nc.vector.tensor_tensor(out=yx[:], in0=y_sb[:], in1=x_sb[:],
                                    op=mybir.AluOpType.mult)
            yx_sum = mpool.tile([1, 1], F32)
            nc.vector.tensor_reduce(out=yx_sum[:], in_=yx[:],
                                    axis=mybir.AxisListType.X,
                                    op=mybir.AluOpType.add)
            loss = mpool.tile([1, 1], F32)
            nc.vector.tensor_tensor(out=loss[:], in0=sp_sum[:], in1=yx_sum[:],
                                    op=mybir.AluOpType.subtract)
            nc.sync.dma_start(out=out_d[:], in_=loss[:])

    nc.compile()
    return nc


def _prep_inputs(token_ids, segment_ids, y_true, embed_table, W_hid, b_hid,
                 W_out, b_out):
    """Host-side shard + bucket + pad. Returns (c_sub, in_maps)."""
    token_ids = np.asarray(token_ids, dtype=np.int64)
    segment_ids = np.asarray(segment_ids, dtype=np.int64)
    y_true = np.asarray(y_true, dtype=np.float32)
    embed_bf16 = np.ascontiguousarray(
        np.asarray(embed_table, dtype=np.float32).astype(BF16_NP))

    # sentence-aligned core boundaries
    bounds = np.searchsorted(segment_ids, np.arange(0, B + 1, SEGS_PER_CORE))
    counts = np.bincount(segment_ids, minlength=B).astype(np.float32)
    recip_all = 1.0 / np.maximum(counts, 1.0)

    # bucket tokens per (core, group, quarter)
    per_core = []
    c_max = 0
    for c in range(N_CORES):
        lo, hi = bounds[c], bounds[c + 1]
        tid = token_ids[lo:hi]
        seg_loc = segment_ids[lo:hi] - c * SEGS_PER_CORE
        starts = np.asarray(GROUP_STARTS, dtype=np.int64)
        grp = np.searchsorted(starts[1:], seg_loc, side="right")
        seg_in_grp = (seg_loc - starts[grp]).astype(np.float32)
        q = tid // VQ
        loc_idx = (tid - q * VQ).astype(np.int64)
        subs = []
        for g in range(N_GROUPS):
            for qq in range(N_QUARTERS):
                sel = (grp == g) & (q == qq)
                li, sg = loc_idx[sel], seg_in_grp[sel]
                order = np.argsort(li, kind="stable")
                subs.append((li[order], sg[order]))
                c_max = max(c_max, int(sel.sum()))
        per_core.append(subs)

    c_sub = ((c_max + 127) // 128) * 128
    n_tiles = c_sub // 128
    n_sub = N_GROUPS * N_QUARTERS
    sb_max = [0] * n_sub
    for c in range(N_CORES):
        for sbi, (li, sg) in enumerate(per_core[c]):
            sb_max[sbi] = max(sb_max[sbi], li.shape[0])
    tiles_sb = tuple((m + 127) // 128 for m in sb_max)

    iota = np.broadcast_to(np.arange(128, dtype=np.float32),
                           (128, 128)).astype(BF16_NP)
    in_maps = []
    for c in range(N_CORES):
        idx_arr = np.zeros((128, n_sub * (c_sub // 16)), dtype=np.int16)
        seg_arr = np.full((128, n_sub * n_tiles), -1.0, dtype=BF16_NP)
        for sbi, (li, sg) in enumerate(per_core[c]):
            n = li.shape[0]
            ip = np.zeros(c_sub, dtype=np.int16)
            ip[:n] = li
            sp = np.full(c_sub, -1.0, dtype=np.float32)
            sp[:n] = sg
            wrapped = ip.reshape(c_sub // 16, 16).T  # [16, c_sub//16]
            idx_arr[:, sbi * (c_sub // 16) : (sbi + 1) * (c_sub // 16)] = (
                np.tile(wrapped, (8, 1))
            )
            seg_arr[:, sbi * n_tiles : (sbi + 1) * n_tiles] = (
                sp.reshape(n_tiles, 128).T.astype(BF16_NP)
            )
        recip_c = np.ones((128, N_GROUPS), dtype=np.float32)
        for g in range(N_GROUPS):
            gstart, gsize = GROUP_STARTS[g], GROUP_SEGS[g]
            recip_c[:gsize, g] = recip_all[
                c * SEGS_PER_CORE + gstart : c * SEGS_PER_CORE + gstart + gsize
            ]
        in_maps.append({
            "embed": embed_bf16,
            "idx": idx_arr,
            "seg": seg_arr,
            "recip": recip_c,
            "iota": iota,
            "y": np.ascontiguousarray(
                y_true[c * SEGS_PER_CORE : (c + 1) * SEGS_PER_CORE]
            ).reshape(1, SEGS_PER_CORE),
            "w_hid": np.ascontiguousarray(np.asarray(W_hid, dtype=np.float32)),
            "b_hid": np.asarray(b_hid, dtype=np.float32).reshape(H, 1),
            "w_out": np.ascontiguousarray(np.asarray(W_out, dtype=np.float32)),
            "b_out": np.asarray(b_out, dtype=np.float32).reshape(1, 1),
        })
    return c_sub, tiles_sb, in_maps


_CACHE = {}


def _get_nc(c_sub, tiles_sb):
    key = (c_sub, tiles_sb)
    nc = _CACHE.get(key)
    if nc is None:
        nc = bacc.Bacc("TRN2", target_bir_lowering=False, debug=False,
                       num_devices=N_CORES, num_swdge_queues=N_QUARTERS)
        _build(nc, c_sub, tiles_sb)
        _CACHE[key] = nc
    return nc


def kernel(token_ids, segment_ids, y_true, embed_table, W_hid, b_hid, W_out,
           b_out, _trace=False, _trace_kwargs=None):
    c_sub, tiles_sb, in_maps = _prep_inputs(token_ids, segment_ids, y_true,
                                            embed_table, W_hid, b_hid, W_out,
                                            b_out)
    nc = _get_nc(c_sub, tiles_sb)
    res = run_bass_kernel_spmd(nc, in_maps, core_ids=list(range(N_CORES)),
                               trace=_trace, **(_trace_kwargs or {}))
    total = np.float64(0.0)
    for r in res.results:
        total += np.float64(r["out"][0, 0])
    out = np.array(np.float32(total))
    if _trace:
        return out, res
    return out
